# revision 12
# baseline (speedup 1.0000x reference)
"""SSD ConfidenceLoss on 8 TRN2 NeuronCores (Bass/Tile).

Math
----
loss[b,d,c] = -gts * log_softmax(predicts);  per box:
  s        = sum_c exp(p_c)      (|p| < ~6, no max-sub needed)
  lse      = ln(s)
  gps      = sum_c g_c * p_c     (= p at the label for one-hot g)
  pgl      = p_last * g_last
  box_loss = lse * gsum - gps    (gsum == 1 one-hot)
  neg_val  = g_last * (lse - p_last) = lse * g_last - pgl
pos_loss = sum(box_loss * pos);  N = sum(pos)
neg_loss = sum of top-neg_num of where(pos, -inf, neg_val),
           neg_num = min(3N, total-N).
Every neg_val >= 0 and pos boxes are masked to 0 here, so the top-k sum
equals S = sum(masked) whenever nnz = count(masked > 0) <= neg_num; the
host checks that (N from pos_indicator, S/nnz from the negvals output)
and falls back to an exact partition otherwise.

Device layout (per core, SPMD, no collectives)
----------------------------------------------
Boxes per core 69,856, zero-padded to 70,656 = 6 groups x 92 chunks x
128.  Transposed "class-on-partition" layout [126, 11776]: SBUF row
21g + c, column j = 128*ch + p holds box m = 11776 g + j, class c.
 - predicts bf16 [126, 11776] (host cast + transpose), 4 tiles on the
   sync HWDGE ring; gts uint8 on gpsimd SWDGE (casts to bf16 on load);
   consts + outputs on the scalar HWDGE ring (3 parallel DMA paths).
 - ACT: exp per tile; both epilogue Lns at the end (2 table loads).
 - DVE: pg = p * g (bf16 2x), PSUM->SBUF copies, per-box epilogue.
 - PE: per 128-column chunk, the DATA (e or pg chunk) is the stationary
   operand and a tiny constant weight tile streams as the moving
   operand: W[:, 0:6] = per-group ones-blocks (class sums), W[:, 6:12] =
   last-class selector.  Output [128 boxes, 6/12] lands box-major in
   PSUM: s/gps/pgl for 128 boxes with NO partition reshuffle.
Per-box natural layout [128, 552]: partition p = j % 128, col =
6*ch + g.  pos and gts[..., -1] come from the host in this layout as
f32; the epilogue computes pos_loss partials (accum_out) and the
masked neg values (negvals output); N/S/nnz are host-side.
"""

import os
import sys

import numpy as np
import ml_dtypes

for _p in ("/opt/trn_rl_repo",):
    if _p not in sys.path:
        sys.path.append(_p)

B, D, C = 64, 8732, 21
NEG_FACTOR = 3
N_CORES = 8

G = 6                       # box groups packed on partitions (6*21 = 126)
KP = G * C                  # 126 contraction partitions
CHUNK = 128                 # boxes per PE chunk (= lhsT free dim)
NCH = 92                    # chunks per core
NB = NCH * CHUNK            # 11776 boxes per group (padded)
BOXES_PER_CORE = B * D // N_CORES       # 69,856
BOXES_PAD = G * NB                      # 70,656
NCOLS = NCH * G             # 552 natural per-box columns
T = 4                       # device tiles (DMA/compute granularity)
CPT = NCH // T              # 23 chunks per tile (one psum group)
JT = NB // T                # 2944 columns per tile
EH = 2                      # epilogue halves (both after the tile loop)

PG_ENGINE = os.environ.get("KPG", "vector")  # vector | gpsimd

_CACHE = {}


def _build(onehot=True):
    key = ("nc", onehot, PG_ENGINE)
    if key in _CACHE:
        return _CACHE[key]

    import concourse.mybir as mybir
    import concourse.tile as tile
    from concourse import bacc

    f32 = mybir.dt.float32
    bf16 = mybir.dt.bfloat16
    u8 = mybir.dt.uint8

    nc = bacc.Bacc("TRN2", target_bir_lowering=False, debug=False,
                   num_devices=N_CORES)

    pred = nc.dram_tensor("predicts", [KP, NB], bf16,
                          kind="ExternalInput").ap()
    gts = nc.dram_tensor("gts", [KP, NB], u8 if onehot else bf16,
                         kind="ExternalInput").ap()
    posf = nc.dram_tensor("posf", [CHUNK, NCOLS], f32, kind="ExternalInput").ap()
    gl = nc.dram_tensor("gl", [CHUNK, NCOLS], f32, kind="ExternalInput").ap()
    wmat = nc.dram_tensor("wmat", [KP, 12], bf16, kind="ExternalInput").ap()
    stats = nc.dram_tensor("stats", [CHUNK, EH], f32, kind="ExternalOutput").ap()
    negvals = nc.dram_tensor("negvals", [CHUNK, NCOLS], f32,
                             kind="ExternalOutput").ap()

    Exp = mybir.ActivationFunctionType.Exp
    Ln = mybir.ActivationFunctionType.Ln
    mult = mybir.AluOpType.mult
    add = mybir.AluOpType.add

    pg_eng = nc.gpsimd if PG_ENGINE == "gpsimd" else nc.vector

    with tile.TileContext(nc) as tc:
        with (
            tc.tile_pool(name="big", bufs=2) as big,
            tc.tile_pool(name="small", bufs=2) as small,
            tc.tile_pool(name="psum", bufs=2, space="PSUM") as psum,
            tc.tile_pool(name="const", bufs=1) as const,
        ):
            w_t = const.tile([KP, 12], bf16)
            nc.scalar.dma_start(w_t[:], wmat[:])
            posf_t = const.tile([CHUNK, NCOLS], f32)
            nc.scalar.dma_start(posf_t[:], posf[:])
            gl_t = const.tile([CHUNK, NCOLS], f32)
            nc.scalar.dma_start(gl_t[:], gl[:])
            # per-box stats, chunk-major: [128, NCH, 18] -> s | gps | pgl
            nat = const.tile([CHUNK, NCH * 18], f32)
            stats_t = const.tile([CHUNK, EH], f32)
            if not onehot:
                gsum_nat = const.tile([CHUNK, NCH * 6], f32)

            for t in range(T):
                p_bf = big.tile([KP, JT], bf16, tag="p")
                nc.sync.dma_start(p_bf[:], pred[:, t * JT:(t + 1) * JT])
                g_bf = big.tile([KP, JT], bf16, tag="g")
                if onehot:  # SWDGE casts u8 -> bf16 on the way in
                    nc.gpsimd.dma_start(g_bf[:], gts[:, t * JT:(t + 1) * JT])
                else:
                    nc.sync.dma_start(g_bf[:], gts[:, t * JT:(t + 1) * JT])

                e_bf = big.tile([KP, JT], bf16, tag="e")
                nc.scalar.activation(e_bf[:], p_bf[:], Exp)
                pg_bf = big.tile([KP, JT], bf16, tag="pg")
                pg_eng.tensor_mul(pg_bf[:], p_bf[:], g_bf[:])

                ps = psum.tile([CHUNK, CPT * 18], f32, tag="ps")
                if not onehot:
                    ps_g = psum.tile([CHUNK, CPT * 6], f32, tag="psg")
                for k in range(CPT):
                    col = k * CHUNK
                    nc.tensor.matmul(ps[:, k * 18:k * 18 + 6],
                                     e_bf[:, col:col + CHUNK],
                                     w_t[:, 0:6])
                    nc.tensor.matmul(ps[:, k * 18 + 6:k * 18 + 18],
                                     pg_bf[:, col:col + CHUNK],
                                     w_t[:, 0:12])
                    if not onehot:
                        nc.tensor.matmul(ps_g[:, k * 6:k * 6 + 6],
                                         g_bf[:, col:col + CHUNK],
                                         w_t[:, 0:6])
                gi = t * CPT * 18
                nc.vector.tensor_copy(nat[:, gi:gi + CPT * 18], ps[:])
                if not onehot:
                    gg = t * CPT * 6
                    nc.vector.tensor_copy(gsum_nat[:, gg:gg + CPT * 6],
                                          ps_g[:])

            # epilogue: two halves, both after the tile loop so the ACT
            # stream is exp x4, [ln table], ln x2 (no table thrash)
            QH = NCH // EH                  # 46 chunks per half
            for h in range(EH):
                nb0 = h * QH * 18
                v = nat[:, nb0:nb0 + QH * 18].rearrange(
                    "p (q s) -> p q s", s=18)
                c0 = h * QH * 6
                pos_h = posf_t[:, c0:c0 + QH * 6]
                gl_h = gl_t[:, c0:c0 + QH * 6]

                lse = small.tile([CHUNK, QH * 6], f32, tag="lse")
                lse3 = lse[:].rearrange("p (q s) -> p q s", s=6)
                nc.scalar.activation(lse3, v[:, :, 0:6], Ln)

                bl = small.tile([CHUNK, QH * 6], f32, tag="bl")
                bl3 = bl[:].rearrange("p (q s) -> p q s", s=6)
                if onehot:
                    nc.vector.tensor_sub(bl3, lse3, v[:, :, 6:12])
                else:
                    gv = gsum_nat[:, c0:c0 + QH * 6].rearrange(
                        "p (q s) -> p q s", s=6)
                    t1 = small.tile([CHUNK, QH * 6], f32, tag="t1")
                    t13 = t1[:].rearrange("p (q s) -> p q s", s=6)
                    nc.vector.tensor_mul(t13, lse3, gv)
                    nc.vector.tensor_sub(bl3, t13, v[:, :, 6:12])
                # pos_loss partial (tensor_tensor_reduce is broken on this
                # HW/runtime: NRT_EXEC_UNIT_UNRECOVERABLE; STT works)
                prod = small.tile([CHUNK, QH * 6], f32, tag="prod")
                nc.vector.scalar_tensor_tensor(
                    prod[:], bl[:], 1.0, pos_h, op0=mult, op1=mult,
                    accum_out=stats_t[:, h:h + 1])

                t0 = small.tile([CHUNK, QH * 6], f32, tag="t0")
                nc.vector.tensor_mul(t0[:], lse[:], gl_h)
                nn = small.tile([CHUNK, QH * 6], f32, tag="nn")
                nn3 = nn[:].rearrange("p (q s) -> p q s", s=6)
                t03 = t0[:].rearrange("p (q s) -> p q s", s=6)
                nc.vector.tensor_sub(nn3, t03, v[:, :, 12:18])
                notf = small.tile([CHUNK, QH * 6], f32, tag="notf")
                nc.vector.tensor_scalar(notf[:], pos_h, -1.0, 1.0,
                                        op0=mult, op1=add)
                masked = small.tile([CHUNK, QH * 6], f32, tag="masked")
                nc.vector.tensor_mul(masked[:], nn[:], notf[:])
                nc.scalar.dma_start(negvals[:, c0:c0 + QH * 6], masked[:])

            nc.scalar.dma_start(stats[:], stats_t[:])

    nc.compile()
    _CACHE[key] = nc
    return nc


def _gts_is_onehot(gts):
    g = np.asarray(gts)
    if ((g != 0.0) & (g != 1.0)).any():
        return False
    return bool((g.sum(-1) == 1.0).all())


def _perm_box_major(flat_core):
    """[BOXES_PAD]-flat (box-major) -> [128, 552] natural (p = j%128,
    col = 6*ch + g)."""
    return np.ascontiguousarray(
        flat_core.reshape(G, NCH, CHUNK).transpose(2, 1, 0).reshape(
            CHUNK, NCOLS))


def _unperm_negvals(nv):
    """[128, 552] natural -> [BOXES_PAD] box-major flat."""
    return nv.reshape(CHUNK, NCH, G).transpose(2, 1, 0).reshape(-1)


def _shard_inputs(predicts, gts, pos_indicator, onehot=True):
    """Full inputs -> 8 per-core maps in the transposed device layout."""
    bf16 = ml_dtypes.bfloat16
    pred_flat = np.asarray(predicts, dtype=np.float32).reshape(-1, C)
    gts_flat = np.asarray(gts, dtype=np.float32).reshape(-1, C)
    pos_flat = np.asarray(pos_indicator).reshape(-1)
    g_dt = np.uint8 if onehot else bf16

    w = np.zeros((KP, 12), dtype=bf16)
    for g in range(G):
        w[g * C:(g + 1) * C, g] = 1.0
        w[g * C + C - 1, 6 + g] = 1.0

    in_maps = []
    for i in range(N_CORES):
        b0 = i * BOXES_PER_CORE
        pe = np.zeros((BOXES_PAD, C), dtype=np.float32)
        pe[:BOXES_PER_CORE] = pred_flat[b0:b0 + BOXES_PER_CORE]
        ge = np.zeros((BOXES_PAD, C), dtype=g_dt)
        ge[:BOXES_PER_CORE] = gts_flat[b0:b0 + BOXES_PER_CORE]
        po = np.zeros(BOXES_PAD, dtype=np.float32)
        po[:BOXES_PER_CORE] = pos_flat[b0:b0 + BOXES_PER_CORE]

        # [BOXES_PAD, C] -> [G, NCH, CHUNK, C] -> [G, C, NCH, CHUNK] -> [126, NB]
        pt = pe.astype(bf16).reshape(G, NCH, CHUNK, C).transpose(
            0, 3, 1, 2).reshape(KP, NB)
        gt = ge.reshape(G, NCH, CHUNK, C).transpose(0, 3, 1, 2).reshape(KP, NB)
        glv = ge[:, C - 1].astype(np.float32)

        in_maps.append({
            "predicts": np.ascontiguousarray(pt),
            "gts": np.ascontiguousarray(gt),
            "posf": _perm_box_major(po),
            "gl": _perm_box_major(glv),
            "wmat": w,
        })
    return in_maps


def _combine(results, pos_indicator):
    """Host combine: pos_loss from device stats; N from the input mask;
    S/nnz (and the exact fallback) from the negvals output."""
    N = float(np.asarray(pos_indicator).sum())
    pos_loss = 0.0
    S = 0.0
    nnz = 0.0
    vals_per_core = []
    for r in results:
        pos_loss += r["stats"].astype(np.float64).sum()
        nv = _unperm_negvals(r["negvals"].astype(np.float64))[:BOXES_PER_CORE]
        vals_per_core.append(nv)
        S += nv.sum()
        nnz += (nv > 0).sum()

    total = B * D
    neg_num = min(NEG_FACTOR * N, total - N)
    if nnz <= neg_num:
        neg_loss = S
    else:
        vals = np.concatenate(vals_per_core)
        k = int(round(neg_num))
        neg_loss = np.partition(vals, len(vals) - k)[len(vals) - k:].sum()

    return np.float32((pos_loss + neg_loss) / N)


def kernel(predicts, gts, pos_indicator):
    from concourse.bass_utils import run_bass_kernel_spmd

    onehot = _gts_is_onehot(gts)
    nc = _build(onehot=onehot)
    in_maps = _shard_inputs(predicts, gts, pos_indicator, onehot=onehot)
    res = run_bass_kernel_spmd(nc, in_maps, core_ids=list(range(N_CORES)))
    return _combine(res.results, pos_indicator)


# revision 13
# speedup vs baseline: 1.0539x; 1.0539x over previous
"""SSD ConfidenceLoss on 8 TRN2 NeuronCores (Bass/Tile).

Math
----
loss[b,d,c] = -gts * log_softmax(predicts);  per box:
  s        = sum_c exp(p_c)      (|p| < ~6, no max-sub needed)
  lse      = ln(s)
  gps      = sum_c g_c * p_c     (= p at the label for one-hot g)
  pgl      = p_last * g_last
  box_loss = lse * gsum - gps    (gsum == 1 one-hot)
  neg_val  = g_last * (lse - p_last) = lse * g_last - pgl
pos_loss = sum(box_loss * pos);  N = sum(pos)
neg_loss = sum of top-neg_num of where(pos, -inf, neg_val),
           neg_num = min(3N, total-N).
Every neg_val >= 0 and pos boxes are masked to 0 here, so the top-k sum
equals S = sum(masked) whenever nnz = count(masked > 0) <= neg_num; the
host checks that (N from pos_indicator, S/nnz from the negvals output)
and falls back to an exact partition otherwise.

Device layout (per core, SPMD, no collectives)
----------------------------------------------
Boxes per core 69,856, zero-padded to 70,656 = 6 groups x 92 chunks x
128.  Transposed "class-on-partition" layout [126, 11776]: SBUF row
21g + c, column j = 128*ch + p holds box m = 11776 g + j, class c.
 - predicts bf16 [126, 11776] (host cast + transpose), 4 tiles on the
   sync HWDGE ring; gts uint8 on gpsimd SWDGE (casts to bf16 on load);
   consts + outputs on the scalar HWDGE ring (3 parallel DMA paths).
 - ACT: exp per tile; both epilogue Lns at the end (2 table loads).
 - DVE: pg = p * g (bf16 2x), PSUM->SBUF copies, per-box epilogue.
 - PE: per 128-column chunk, the DATA (e or pg chunk) is the stationary
   operand and a tiny constant weight tile streams as the moving
   operand: W[:, 0:6] = per-group ones-blocks (class sums), W[:, 6:12] =
   last-class selector.  Output [128 boxes, 6/12] lands box-major in
   PSUM: s/gps/pgl for 128 boxes with NO partition reshuffle.
Per-box natural layout [128, 552]: partition p = j % 128, col =
6*ch + g.  pos and gts[..., -1] come from the host in this layout as
f32; the epilogue computes pos_loss partials (accum_out) and the
masked neg values (negvals output); N/S/nnz are host-side.
"""

import os
import sys

import numpy as np
import ml_dtypes

for _p in ("/opt/trn_rl_repo",):
    if _p not in sys.path:
        sys.path.append(_p)

B, D, C = 64, 8732, 21
NEG_FACTOR = 3
N_CORES = 8

G = 6                       # box groups packed on partitions (6*21 = 126)
KP = G * C                  # 126 contraction partitions
CHUNK = 128                 # boxes per PE chunk (= lhsT free dim)
NCH = 92                    # chunks per core
NB = NCH * CHUNK            # 11776 boxes per group (padded)
BOXES_PER_CORE = B * D // N_CORES       # 69,856
BOXES_PAD = G * NB                      # 70,656
NCOLS = NCH * G             # 552 natural per-box columns
T = 2                       # DMA tiles (fat 11.8KB descriptors)
HT = 2                      # compute half-tiles per DMA tile
NG = T * HT                 # 4 psum groups
CPG = NCH // NG             # 23 chunks per psum group
JT = NB // T                # 5888 columns per DMA tile
JH = JT // HT               # 2944 columns per compute half-tile
EH = 2                      # epilogue halves (both after the tile loop)

PG_ENGINE = os.environ.get("KPG", "vector")  # vector | gpsimd

_CACHE = {}


def _build(onehot=True):
    key = ("nc", onehot, PG_ENGINE)
    if key in _CACHE:
        return _CACHE[key]

    import concourse.mybir as mybir
    import concourse.tile as tile
    from concourse import bacc

    f32 = mybir.dt.float32
    bf16 = mybir.dt.bfloat16
    u8 = mybir.dt.uint8

    nc = bacc.Bacc("TRN2", target_bir_lowering=False, debug=False,
                   num_devices=N_CORES)

    pred = nc.dram_tensor("predicts", [KP, NB], bf16,
                          kind="ExternalInput").ap()
    gts = nc.dram_tensor("gts", [KP, NB], u8 if onehot else bf16,
                         kind="ExternalInput").ap()
    posf = nc.dram_tensor("posf", [CHUNK, NCOLS], bf16, kind="ExternalInput").ap()
    gl = nc.dram_tensor("gl", [CHUNK, NCOLS], bf16, kind="ExternalInput").ap()
    wmat = nc.dram_tensor("wmat", [KP, 12], bf16, kind="ExternalInput").ap()
    stats = nc.dram_tensor("stats", [CHUNK, EH], f32, kind="ExternalOutput").ap()
    negvals = nc.dram_tensor("negvals", [CHUNK, NCOLS], bf16,
                             kind="ExternalOutput").ap()

    Exp = mybir.ActivationFunctionType.Exp
    Ln = mybir.ActivationFunctionType.Ln
    mult = mybir.AluOpType.mult
    add = mybir.AluOpType.add

    pg_eng = nc.gpsimd if PG_ENGINE == "gpsimd" else nc.vector

    with tile.TileContext(nc) as tc:
        with (
            tc.tile_pool(name="big", bufs=2) as big,
            tc.tile_pool(name="half", bufs=2) as half,
            tc.tile_pool(name="small", bufs=2) as small,
            tc.tile_pool(name="psum", bufs=2, space="PSUM") as psum,
            tc.tile_pool(name="const", bufs=1) as const,
        ):
            # consts ride the gpsimd SWDGE queue: the scalar-ring DMA
            # issues would otherwise block the ACT queue ahead of exp
            w_t = const.tile([KP, 12], bf16)
            nc.gpsimd.dma_start(w_t[:], wmat[:])
            posf_t = const.tile([CHUNK, NCOLS], bf16)
            nc.gpsimd.dma_start(posf_t[:], posf[:])
            gl_t = const.tile([CHUNK, NCOLS], bf16)
            nc.gpsimd.dma_start(gl_t[:], gl[:])
            # per-box stats, chunk-major: [128, NCH, 18] -> s | gps | pgl
            nat = const.tile([CHUNK, NCH * 18], f32)
            stats_t = const.tile([CHUNK, EH], f32)
            if not onehot:
                gsum_nat = const.tile([CHUNK, NCH * 6], f32)

            for t in range(T):
                p_bf = big.tile([KP, JT], bf16, tag="p")
                nc.sync.dma_start(p_bf[:], pred[:, t * JT:(t + 1) * JT])
                g_bf = big.tile([KP, JT], bf16, tag="g")
                if onehot:  # SWDGE casts u8 -> bf16 on the way in
                    nc.gpsimd.dma_start(g_bf[:], gts[:, t * JT:(t + 1) * JT])
                else:
                    nc.sync.dma_start(g_bf[:], gts[:, t * JT:(t + 1) * JT])

                for ht in range(HT):
                    hb = ht * JH
                    e_bf = half.tile([KP, JH], bf16, tag="e")
                    nc.scalar.activation(e_bf[:], p_bf[:, hb:hb + JH], Exp)
                    pg_bf = half.tile([KP, JH], bf16, tag="pg")
                    pg_eng.tensor_mul(pg_bf[:], p_bf[:, hb:hb + JH],
                                      g_bf[:, hb:hb + JH])

                    ps = psum.tile([CHUNK, CPG * 18], f32, tag="ps")
                    if not onehot:
                        ps_g = psum.tile([CHUNK, CPG * 6], f32, tag="psg")
                    for k in range(CPG):
                        col = k * CHUNK
                        nc.tensor.matmul(ps[:, k * 18:k * 18 + 6],
                                         e_bf[:, col:col + CHUNK],
                                         w_t[:, 0:6])
                        nc.tensor.matmul(ps[:, k * 18 + 6:k * 18 + 18],
                                         pg_bf[:, col:col + CHUNK],
                                         w_t[:, 0:12])
                        if not onehot:
                            nc.tensor.matmul(ps_g[:, k * 6:k * 6 + 6],
                                             g_bf[:, hb + col:hb + col + CHUNK],
                                             w_t[:, 0:6])
                    gi = (t * HT + ht) * CPG * 18
                    nc.vector.tensor_copy(nat[:, gi:gi + CPG * 18], ps[:])
                    if not onehot:
                        gg = (t * HT + ht) * CPG * 6
                        nc.vector.tensor_copy(gsum_nat[:, gg:gg + CPG * 6],
                                              ps_g[:])

            # epilogue: two halves, both after the tile loop so the ACT
            # stream is exp x4, [ln table], ln x2 (no table thrash)
            QH = NCH // EH                  # 46 chunks per half
            for h in range(EH):
                nb0 = h * QH * 18
                v = nat[:, nb0:nb0 + QH * 18].rearrange(
                    "p (q s) -> p q s", s=18)
                c0 = h * QH * 6
                pos_h = posf_t[:, c0:c0 + QH * 6]
                gl_h = gl_t[:, c0:c0 + QH * 6]

                lse = small.tile([CHUNK, QH * 6], f32, tag="lse")
                lse3 = lse[:].rearrange("p (q s) -> p q s", s=6)
                nc.scalar.activation(lse3, v[:, :, 0:6], Ln)

                bl = small.tile([CHUNK, QH * 6], f32, tag="bl")
                bl3 = bl[:].rearrange("p (q s) -> p q s", s=6)
                if onehot:
                    nc.vector.tensor_sub(bl3, lse3, v[:, :, 6:12])
                else:
                    gv = gsum_nat[:, c0:c0 + QH * 6].rearrange(
                        "p (q s) -> p q s", s=6)
                    t1 = small.tile([CHUNK, QH * 6], f32, tag="t1")
                    t13 = t1[:].rearrange("p (q s) -> p q s", s=6)
                    nc.vector.tensor_mul(t13, lse3, gv)
                    nc.vector.tensor_sub(bl3, t13, v[:, :, 6:12])
                # pos_loss partial (tensor_tensor_reduce is broken on this
                # HW/runtime: NRT_EXEC_UNIT_UNRECOVERABLE; STT works)
                prod = small.tile([CHUNK, QH * 6], f32, tag="prod")
                nc.vector.scalar_tensor_tensor(
                    prod[:], bl[:], 1.0, pos_h, op0=mult, op1=mult,
                    accum_out=stats_t[:, h:h + 1])

                # raw neg_val = lse*gl - pgl; positives are zeroed on the
                # host (it has the pos mask), so no device-side masking
                t0 = small.tile([CHUNK, QH * 6], f32, tag="t0")
                nc.vector.tensor_mul(t0[:], lse[:], gl_h)
                nn = small.tile([CHUNK, QH * 6], bf16, tag="nn")
                nn3 = nn[:].rearrange("p (q s) -> p q s", s=6)
                t03 = t0[:].rearrange("p (q s) -> p q s", s=6)
                nc.vector.tensor_sub(nn3, t03, v[:, :, 12:18])
                nc.scalar.dma_start(negvals[:, c0:c0 + QH * 6], nn[:])

            nc.scalar.dma_start(stats[:], stats_t[:])

    nc.compile()
    _CACHE[key] = nc
    return nc


def _gts_is_onehot(gts):
    g = np.asarray(gts)
    if ((g != 0.0) & (g != 1.0)).any():
        return False
    return bool((g.sum(-1) == 1.0).all())


def _perm_box_major(flat_core):
    """[BOXES_PAD]-flat (box-major) -> [128, 552] natural (p = j%128,
    col = 6*ch + g)."""
    return np.ascontiguousarray(
        flat_core.reshape(G, NCH, CHUNK).transpose(2, 1, 0).reshape(
            CHUNK, NCOLS))


def _unperm_negvals(nv):
    """[128, 552] natural -> [BOXES_PAD] box-major flat."""
    return nv.reshape(CHUNK, NCH, G).transpose(2, 1, 0).reshape(-1)


def _shard_inputs(predicts, gts, pos_indicator, onehot=True):
    """Full inputs -> 8 per-core maps in the transposed device layout."""
    bf16 = ml_dtypes.bfloat16
    pred_flat = np.asarray(predicts, dtype=np.float32).reshape(-1, C)
    gts_flat = np.asarray(gts, dtype=np.float32).reshape(-1, C)
    pos_flat = np.asarray(pos_indicator).reshape(-1)
    g_dt = np.uint8 if onehot else bf16

    w = np.zeros((KP, 12), dtype=bf16)
    for g in range(G):
        w[g * C:(g + 1) * C, g] = 1.0
        w[g * C + C - 1, 6 + g] = 1.0

    in_maps = []
    for i in range(N_CORES):
        b0 = i * BOXES_PER_CORE
        pe = np.zeros((BOXES_PAD, C), dtype=np.float32)
        pe[:BOXES_PER_CORE] = pred_flat[b0:b0 + BOXES_PER_CORE]
        ge = np.zeros((BOXES_PAD, C), dtype=g_dt)
        ge[:BOXES_PER_CORE] = gts_flat[b0:b0 + BOXES_PER_CORE]
        po = np.zeros(BOXES_PAD, dtype=np.float32)
        po[:BOXES_PER_CORE] = pos_flat[b0:b0 + BOXES_PER_CORE]

        # [BOXES_PAD, C] -> [G, NCH, CHUNK, C] -> [G, C, NCH, CHUNK] -> [126, NB]
        pt = pe.astype(bf16).reshape(G, NCH, CHUNK, C).transpose(
            0, 3, 1, 2).reshape(KP, NB)
        gt = ge.reshape(G, NCH, CHUNK, C).transpose(0, 3, 1, 2).reshape(KP, NB)
        glv = ge[:, C - 1].astype(np.float32)

        in_maps.append({
            "predicts": np.ascontiguousarray(pt),
            "gts": np.ascontiguousarray(gt),
            "posf": _perm_box_major(po).astype(bf16),
            "gl": _perm_box_major(glv).astype(bf16),
            "wmat": w,
        })
    return in_maps


def _combine(results, pos_indicator):
    """Host combine: pos_loss from device stats; N from the input mask;
    S/nnz (and the exact fallback) from the negvals output."""
    pos_flat = np.asarray(pos_indicator).reshape(-1)
    N = float(pos_flat.sum())
    pos_loss = 0.0
    S = 0.0
    nnz = 0.0
    vals_per_core = []
    for i, r in enumerate(results):
        pos_loss += r["stats"].astype(np.float64).sum()
        nv = _unperm_negvals(r["negvals"].astype(np.float64))[:BOXES_PER_CORE]
        b0 = i * BOXES_PER_CORE
        nv[pos_flat[b0:b0 + BOXES_PER_CORE]] = 0.0
        vals_per_core.append(nv)
        S += nv.sum()
        nnz += (nv > 0).sum()

    total = B * D
    neg_num = min(NEG_FACTOR * N, total - N)
    if nnz <= neg_num:
        neg_loss = S
    else:
        vals = np.concatenate(vals_per_core)
        k = int(round(neg_num))
        neg_loss = np.partition(vals, len(vals) - k)[len(vals) - k:].sum()

    return np.float32((pos_loss + neg_loss) / N)


def kernel(predicts, gts, pos_indicator):
    from concourse.bass_utils import run_bass_kernel_spmd

    onehot = _gts_is_onehot(gts)
    nc = _build(onehot=onehot)
    in_maps = _shard_inputs(predicts, gts, pos_indicator, onehot=onehot)
    res = run_bass_kernel_spmd(nc, in_maps, core_ids=list(range(N_CORES)))
    return _combine(res.results, pos_indicator)


# revision 14
# speedup vs baseline: 1.0955x; 1.0395x over previous
"""SSD ConfidenceLoss on 8 TRN2 NeuronCores (Bass/Tile).

Math
----
loss[b,d,c] = -gts * log_softmax(predicts);  per box:
  s        = sum_c exp(p_c)      (|p| < ~6, no max-sub needed)
  lse      = ln(s)
  gps      = sum_c g_c * p_c     (= p at the label for one-hot g)
  pgl      = p_last * g_last
  box_loss = lse * gsum - gps    (gsum == 1 one-hot)
  neg_val  = g_last * (lse - p_last) = lse * g_last - pgl
pos_loss = sum(box_loss * pos);  N = sum(pos)
neg_loss = sum of top-neg_num of where(pos, -inf, neg_val),
           neg_num = min(3N, total-N).
Every neg_val >= 0 and pos boxes are masked to 0 here, so the top-k sum
equals S = sum(masked) whenever nnz = count(masked > 0) <= neg_num; the
host checks that (N from pos_indicator, S/nnz from the negvals output)
and falls back to an exact partition otherwise.

Device layout (per core, SPMD, no collectives)
----------------------------------------------
Boxes per core 69,856, zero-padded to 70,656 = 6 groups x 92 chunks x
128.  Transposed "class-on-partition" layout [126, 11776]: SBUF row
21g + c, column j = 128*ch + p holds box m = 11776 g + j, class c.
 - predicts bf16 [126, 11776] (host cast + transpose), 4 tiles on the
   sync HWDGE ring; gts uint8 on gpsimd SWDGE (casts to bf16 on load);
   consts + outputs on the scalar HWDGE ring (3 parallel DMA paths).
 - ACT: exp per tile; both epilogue Lns at the end (2 table loads).
 - DVE: pg = p * g (bf16 2x), PSUM->SBUF copies, per-box epilogue.
 - PE: per 128-column chunk, the DATA (e or pg chunk) is the stationary
   operand and a tiny constant weight tile streams as the moving
   operand: W[:, 0:6] = per-group ones-blocks (class sums), W[:, 6:12] =
   last-class selector.  Output [128 boxes, 6/12] lands box-major in
   PSUM: s/gps/pgl for 128 boxes with NO partition reshuffle.
Per-box natural layout [128, 552]: partition p = j % 128, col =
6*ch + g.  pos and gts[..., -1] come from the host in this layout as
f32; the epilogue computes pos_loss partials (accum_out) and the
masked neg values (negvals output); N/S/nnz are host-side.
"""

import os
import sys

import numpy as np
import ml_dtypes

for _p in ("/opt/trn_rl_repo",):
    if _p not in sys.path:
        sys.path.append(_p)

B, D, C = 64, 8732, 21
NEG_FACTOR = 3
N_CORES = 8

G = 6                       # box groups packed on partitions (6*21 = 126)
KP = G * C                  # 126 contraction partitions
CHUNK = 128                 # boxes per PE chunk (= lhsT free dim)
NCH = 92                    # chunks per core
NB = NCH * CHUNK            # 11776 boxes per group (padded)
BOXES_PER_CORE = B * D // N_CORES       # 69,856
BOXES_PAD = G * NB                      # 70,656
NCOLS = NCH * G             # 552 natural per-box columns
T = 2                       # DMA tiles (fat 11.8KB descriptors)
HT = 2                      # compute half-tiles per DMA tile
NG = T * HT                 # 4 psum groups
CPG = NCH // NG             # 23 chunks per psum group
JT = NB // T                # 5888 columns per DMA tile
JH = JT // HT               # 2944 columns per compute half-tile
EH = 2                      # epilogue halves (both after the tile loop)

PG_ENGINE = os.environ.get("KPG", "vector")  # vector | gpsimd

_CACHE = {}


def _build(onehot=True):
    key = ("nc", onehot, PG_ENGINE)
    if key in _CACHE:
        return _CACHE[key]

    import concourse.mybir as mybir
    import concourse.tile as tile
    from concourse import bacc

    f32 = mybir.dt.float32
    bf16 = mybir.dt.bfloat16
    u8 = mybir.dt.uint8

    nc = bacc.Bacc("TRN2", target_bir_lowering=False, debug=False,
                   num_devices=N_CORES)

    pred = nc.dram_tensor("predicts", [KP, NB], bf16,
                          kind="ExternalInput").ap()
    gts = nc.dram_tensor("gts", [KP, NB], u8 if onehot else bf16,
                         kind="ExternalInput").ap()
    posf = nc.dram_tensor("posf", [CHUNK, NCOLS], bf16, kind="ExternalInput").ap()
    gl = nc.dram_tensor("gl", [CHUNK, NCOLS], bf16, kind="ExternalInput").ap()
    wmat = nc.dram_tensor("wmat", [KP, 12], bf16, kind="ExternalInput").ap()
    stats = nc.dram_tensor("stats", [CHUNK, EH], f32, kind="ExternalOutput").ap()
    negvals = nc.dram_tensor("negvals", [CHUNK, NCOLS], bf16,
                             kind="ExternalOutput").ap()

    Exp = mybir.ActivationFunctionType.Exp
    Ln = mybir.ActivationFunctionType.Ln
    mult = mybir.AluOpType.mult
    add = mybir.AluOpType.add

    pg_eng = nc.gpsimd if PG_ENGINE == "gpsimd" else nc.vector

    with tile.TileContext(nc) as tc:
        with (
            tc.tile_pool(name="big", bufs=2) as big,
            tc.tile_pool(name="half", bufs=2) as half,
            tc.tile_pool(name="small", bufs=2) as small,
            tc.tile_pool(name="psum", bufs=2, space="PSUM") as psum,
            tc.tile_pool(name="const", bufs=1) as const,
        ):
            # consts ride the gpsimd SWDGE queue: the scalar-ring DMA
            # issues would otherwise block the ACT queue ahead of exp
            w_t = const.tile([KP, 12], bf16)
            nc.gpsimd.dma_start(w_t[:], wmat[:])
            posf_t = const.tile([CHUNK, NCOLS], bf16)
            nc.gpsimd.dma_start(posf_t[:], posf[:])
            gl_t = const.tile([CHUNK, NCOLS], bf16)
            nc.gpsimd.dma_start(gl_t[:], gl[:])
            # per-box stats, chunk-major: [128, NCH, 18] -> s | gps | pgl
            nat = const.tile([CHUNK, NCH * 18], f32)
            stats_t = const.tile([CHUNK, EH], f32)
            if not onehot:
                gsum_nat = const.tile([CHUNK, NCH * 6], f32)

            for t in range(T):
                p_bf = big.tile([KP, JT], bf16, tag="p")
                nc.sync.dma_start(p_bf[:], pred[:, t * JT:(t + 1) * JT])
                # g rides the scalar HWDGE ring uncast (the SWDGE u8->bf16
                # cast DMA runs at ~25 GB/s -- useless); the TT product
                # reads the u8 operand directly
                g_bf = big.tile([KP, JT], u8 if onehot else bf16, tag="g")
                nc.scalar.dma_start(g_bf[:], gts[:, t * JT:(t + 1) * JT])

                for ht in range(HT):
                    hb = ht * JH
                    e_bf = half.tile([KP, JH], bf16, tag="e")
                    nc.scalar.activation(e_bf[:], p_bf[:, hb:hb + JH], Exp)
                    pg_bf = half.tile([KP, JH], bf16, tag="pg")
                    pg_eng.tensor_mul(pg_bf[:], p_bf[:, hb:hb + JH],
                                      g_bf[:, hb:hb + JH])

                    ps = psum.tile([CHUNK, CPG * 18], f32, tag="ps")
                    if not onehot:
                        ps_g = psum.tile([CHUNK, CPG * 6], f32, tag="psg")
                    for k in range(CPG):
                        col = k * CHUNK
                        nc.tensor.matmul(ps[:, k * 18:k * 18 + 6],
                                         e_bf[:, col:col + CHUNK],
                                         w_t[:, 0:6])
                        nc.tensor.matmul(ps[:, k * 18 + 6:k * 18 + 18],
                                         pg_bf[:, col:col + CHUNK],
                                         w_t[:, 0:12])
                        if not onehot:
                            nc.tensor.matmul(ps_g[:, k * 6:k * 6 + 6],
                                             g_bf[:, hb + col:hb + col + CHUNK],
                                             w_t[:, 0:6])
                    gi = (t * HT + ht) * CPG * 18
                    nc.vector.tensor_copy(nat[:, gi:gi + CPG * 18], ps[:])
                    if not onehot:
                        gg = (t * HT + ht) * CPG * 6
                        nc.vector.tensor_copy(gsum_nat[:, gg:gg + CPG * 6],
                                              ps_g[:])

            # epilogue: two halves, both after the tile loop so the ACT
            # stream is exp x4, [ln table], ln x2 (no table thrash)
            QH = NCH // EH                  # 46 chunks per half
            for h in range(EH):
                nb0 = h * QH * 18
                v = nat[:, nb0:nb0 + QH * 18].rearrange(
                    "p (q s) -> p q s", s=18)
                c0 = h * QH * 6
                pos_h = posf_t[:, c0:c0 + QH * 6]
                gl_h = gl_t[:, c0:c0 + QH * 6]

                lse = small.tile([CHUNK, QH * 6], f32, tag="lse")
                lse3 = lse[:].rearrange("p (q s) -> p q s", s=6)
                nc.scalar.activation(lse3, v[:, :, 0:6], Ln)

                bl = small.tile([CHUNK, QH * 6], f32, tag="bl")
                bl3 = bl[:].rearrange("p (q s) -> p q s", s=6)
                if onehot:
                    nc.vector.tensor_sub(bl3, lse3, v[:, :, 6:12])
                else:
                    gv = gsum_nat[:, c0:c0 + QH * 6].rearrange(
                        "p (q s) -> p q s", s=6)
                    t1 = small.tile([CHUNK, QH * 6], f32, tag="t1")
                    t13 = t1[:].rearrange("p (q s) -> p q s", s=6)
                    nc.vector.tensor_mul(t13, lse3, gv)
                    nc.vector.tensor_sub(bl3, t13, v[:, :, 6:12])
                # pos_loss partial (tensor_tensor_reduce is broken on this
                # HW/runtime: NRT_EXEC_UNIT_UNRECOVERABLE; STT works)
                prod = small.tile([CHUNK, QH * 6], f32, tag="prod")
                nc.vector.scalar_tensor_tensor(
                    prod[:], bl[:], 1.0, pos_h, op0=mult, op1=mult,
                    accum_out=stats_t[:, h:h + 1])

                # raw neg_val = lse*gl - pgl; positives are zeroed on the
                # host (it has the pos mask), so no device-side masking
                t0 = small.tile([CHUNK, QH * 6], f32, tag="t0")
                nc.vector.tensor_mul(t0[:], lse[:], gl_h)
                nn = small.tile([CHUNK, QH * 6], bf16, tag="nn")
                nn3 = nn[:].rearrange("p (q s) -> p q s", s=6)
                t03 = t0[:].rearrange("p (q s) -> p q s", s=6)
                nc.vector.tensor_sub(nn3, t03, v[:, :, 12:18])
                nc.scalar.dma_start(negvals[:, c0:c0 + QH * 6], nn[:])

            nc.scalar.dma_start(stats[:], stats_t[:])

    nc.compile()
    _CACHE[key] = nc
    return nc


def _gts_is_onehot(gts):
    g = np.asarray(gts)
    if ((g != 0.0) & (g != 1.0)).any():
        return False
    return bool((g.sum(-1) == 1.0).all())


def _perm_box_major(flat_core):
    """[BOXES_PAD]-flat (box-major) -> [128, 552] natural (p = j%128,
    col = 6*ch + g)."""
    return np.ascontiguousarray(
        flat_core.reshape(G, NCH, CHUNK).transpose(2, 1, 0).reshape(
            CHUNK, NCOLS))


def _unperm_negvals(nv):
    """[128, 552] natural -> [BOXES_PAD] box-major flat."""
    return nv.reshape(CHUNK, NCH, G).transpose(2, 1, 0).reshape(-1)


def _shard_inputs(predicts, gts, pos_indicator, onehot=True):
    """Full inputs -> 8 per-core maps in the transposed device layout."""
    bf16 = ml_dtypes.bfloat16
    pred_flat = np.asarray(predicts, dtype=np.float32).reshape(-1, C)
    gts_flat = np.asarray(gts, dtype=np.float32).reshape(-1, C)
    pos_flat = np.asarray(pos_indicator).reshape(-1)
    g_dt = np.uint8 if onehot else bf16

    w = np.zeros((KP, 12), dtype=bf16)
    for g in range(G):
        w[g * C:(g + 1) * C, g] = 1.0
        w[g * C + C - 1, 6 + g] = 1.0

    in_maps = []
    for i in range(N_CORES):
        b0 = i * BOXES_PER_CORE
        pe = np.zeros((BOXES_PAD, C), dtype=np.float32)
        pe[:BOXES_PER_CORE] = pred_flat[b0:b0 + BOXES_PER_CORE]
        ge = np.zeros((BOXES_PAD, C), dtype=g_dt)
        ge[:BOXES_PER_CORE] = gts_flat[b0:b0 + BOXES_PER_CORE]
        po = np.zeros(BOXES_PAD, dtype=np.float32)
        po[:BOXES_PER_CORE] = pos_flat[b0:b0 + BOXES_PER_CORE]

        # [BOXES_PAD, C] -> [G, NCH, CHUNK, C] -> [G, C, NCH, CHUNK] -> [126, NB]
        pt = pe.astype(bf16).reshape(G, NCH, CHUNK, C).transpose(
            0, 3, 1, 2).reshape(KP, NB)
        gt = ge.reshape(G, NCH, CHUNK, C).transpose(0, 3, 1, 2).reshape(KP, NB)
        glv = ge[:, C - 1].astype(np.float32)

        in_maps.append({
            "predicts": np.ascontiguousarray(pt),
            "gts": np.ascontiguousarray(gt),
            "posf": _perm_box_major(po).astype(bf16),
            "gl": _perm_box_major(glv).astype(bf16),
            "wmat": w,
        })
    return in_maps


def _combine(results, pos_indicator):
    """Host combine: pos_loss from device stats; N from the input mask;
    S/nnz (and the exact fallback) from the negvals output."""
    pos_flat = np.asarray(pos_indicator).reshape(-1)
    N = float(pos_flat.sum())
    pos_loss = 0.0
    S = 0.0
    nnz = 0.0
    vals_per_core = []
    for i, r in enumerate(results):
        pos_loss += r["stats"].astype(np.float64).sum()
        nv = _unperm_negvals(r["negvals"].astype(np.float64))[:BOXES_PER_CORE]
        b0 = i * BOXES_PER_CORE
        nv[pos_flat[b0:b0 + BOXES_PER_CORE]] = 0.0
        vals_per_core.append(nv)
        S += nv.sum()
        nnz += (nv > 0).sum()

    total = B * D
    neg_num = min(NEG_FACTOR * N, total - N)
    if nnz <= neg_num:
        neg_loss = S
    else:
        vals = np.concatenate(vals_per_core)
        k = int(round(neg_num))
        neg_loss = np.partition(vals, len(vals) - k)[len(vals) - k:].sum()

    return np.float32((pos_loss + neg_loss) / N)


def kernel(predicts, gts, pos_indicator):
    from concourse.bass_utils import run_bass_kernel_spmd

    onehot = _gts_is_onehot(gts)
    nc = _build(onehot=onehot)
    in_maps = _shard_inputs(predicts, gts, pos_indicator, onehot=onehot)
    res = run_bass_kernel_spmd(nc, in_maps, core_ids=list(range(N_CORES)))
    return _combine(res.results, pos_indicator)
